# revision 1
# baseline (speedup 1.0000x reference)
"""Trainium2 Bass kernel for the ACTP 2-layer-LSTM rollout (nn_ACTP_68143951119114).

Self-contained: builds a feature-major bf16 Bass/Tile kernel, shards the batch
across 8 NeuronCores (pure data parallelism), runs via run_bass_kernel_spmd,
and gathers/transposes the result on the host.

Design (per-core, Bc = B/8 = 4096, batch chunks of 512):
- Activations kept feature-major [feature, batch] so every matmul contracts
  over the partition dim; hidden state h split 128+72 ("lo"/"hi") tiles.
- All weights host-preprocessed: gate rows permuted to [i,f,o,g] x (lo,hi)
  M-tiles, biases folded in as ones-row columns, the action "tiled" structure
  reduced to 12 effective rows; everything cast to bf16 (TensorE streams
  2 cols/cycle for bf16).
- LSTM gate matmuls accumulate into PSUM slabs ([i,f,o] 3 banks + [g] 1 bank
  per half); ScalarE applies sigmoid/tanh straight out of PSUM into fp32 SBUF
  gate tiles; VectorE does the cell update (c stays fp32); tanh(c) uses the
  fast fp32 SBUF->SBUF ScalarE path; h is written bf16 for the next matmuls.
- The fc head (tanh fc1+fc2) runs only for steps >= cf-1; its output feeds
  both the DRAM output and the next step's x1 (prediction feedback).
- Chunks are software-pipelined with lstm2/fc trailing lstm1 by 2 chunks so
  PE/ACT/DVE overlap across the batch dimension.
"""
import sys as _sys
for _p in ("/opt/trn_rl_repo", "/root/.axon_site/_ro/trn_rl_repo"):
    if _p not in _sys.path:
        _sys.path.append(_p)

import numpy as np
import ml_dtypes
import contextlib

import concourse.bass as bass
import concourse.tile as tile
from concourse import bacc, mybir

F = mybir.ActivationFunctionType
A = mybir.AluOpType
BF = mybir.dt.bfloat16
F32 = mybir.dt.float32

H, DT, DA = 200, 48, 6
HLO, HHI = 128, 72
NC = 512  # batch chunk for matmul N


def perm_indices():
    # PT gate row ranges in z: i 0:200, f 200:400, g 400:600, o 600:800
    gr = {"i": 0, "f": 200, "o": 600, "g": 400}
    order = ["i", "f", "o", "g"]
    lo = np.concatenate([np.arange(gr[g], gr[g] + HLO) for g in order])
    hi = np.concatenate([np.arange(gr[g] + HLO, gr[g] + H) for g in order])
    return np.concatenate([lo, hi])


def prep_weights(inp):
    """Host-side weight prep. Returns dict name -> np bf16 array."""
    P = perm_indices()
    f32 = np.float32
    Wih1, Whh1 = f32(inp["Wih1"]), f32(inp["Whh1"])
    Wih2, Whh2 = f32(inp["Wih2"]), f32(inp["Whh2"])
    W1, W2 = f32(inp["W1"]), f32(inp["W2"])
    b1, b2 = f32(inp["b1"]), f32(inp["b2"])
    bb1 = f32(inp["bih1"]) + f32(inp["bhh1"])
    bb2 = f32(inp["bih2"]) + f32(inp["bhh2"])

    # z1 rhs pieces: xt1=[x1(48); ones(1)] (49), h1hi (72), h1A (128)
    wz1k0 = np.concatenate([Wih1, bb1[:, None]], 1)[P].T  # [49,800]
    wz1k1 = Whh1[:, HLO:H][P].T  # [72,800]
    wz1k2 = Whh1[:, 0:HLO][P].T  # [128,800]

    # z2 rhs pieces: h1A (128), hs1=[h1_hi(72); act(6); state(6); ones(1)] (85),
    #                h2hi[0:72] (72), h2A (128)
    Wt = Wih2[:, H:H + DT]  # tiled part [800, 48]
    Wact = Wt[:, 0:6] + Wt[:, 12:18] + Wt[:, 24:30] + Wt[:, 36:42]
    Wsta = Wt[:, 6:12] + Wt[:, 18:24] + Wt[:, 30:36] + Wt[:, 42:48]
    wz2k0 = Wih2[:, 0:HLO][P].T  # [128,800]
    wz2k1 = np.concatenate([Wih2[:, HLO:H], Wact, Wsta, bb2[:, None]], 1)[P].T  # [85,800]
    wz2k2 = Whh2[:, HLO:H][P].T  # [72,800]
    wz2k3 = Whh2[:, 0:HLO][P].T  # [128,800]

    # fc1: lp = [h2(200); x1(48)], W1 [200, 248]
    # pieces: h2A: W1[:,0:128]; h2hi[0:73]=[h2_hi;ones]: [W1[:,128:200], b1]; xh1[0:48]: W1[:,200:248]
    wf1k0 = W1[:, 0:HLO].T  # [128,200]
    wf1k1 = np.concatenate([W1[:, HLO:H], b1[:, None]], 1).T  # [73,200]
    wf1k2 = W1[:, H:H + DT].T  # [48,200]
    # fc2: pieces o3A: W2[:,0:128]; o3hi[0:73]=[o3_hi;ones]: [W2[:,128:200], b2]
    wf2k0 = W2[:, 0:HLO].T  # [128,48]
    wf2k1 = W2[:, HLO:H].T  # [72,48]
    wf2k2 = b2[None, :]     # [1,48]

    ws = dict(wz1k0=wz1k0, wz1k1=wz1k1, wz1k2=wz1k2, wz2k0=wz2k0, wz2k1=wz2k1,
              wz2k2=wz2k2, wz2k3=wz2k3, wf1k0=wf1k0, wf1k1=wf1k1, wf1k2=wf1k2,
              wf2k0=wf2k0, wf2k1=wf2k1, wf2k2=wf2k2)
    return {k: np.ascontiguousarray(v.astype(ml_dtypes.bfloat16)) for k, v in ws.items()}


def prep_core_inputs(tactiles, actions, cf, T, core, n_cores):
    """Per-core input shards, feature-major, bf16."""
    B = tactiles.shape[1]
    Bc = B // n_cores
    sl = slice(core * Bc, (core + 1) * Bc)
    bf = ml_dtypes.bfloat16
    # tact: [cf, 48, Bc]
    tact = np.ascontiguousarray(
        np.transpose(tactiles[:cf, sl, :], (0, 2, 1)).astype(bf))
    # actstate: [T-1, 12, Bc]: rows = [actions[s+1].T ; actions[0].T]
    at = np.transpose(actions[:, sl, :], (0, 2, 1))  # [T, 6, Bc]
    acts = np.concatenate(
        [at[1:T], np.broadcast_to(at[0:1], (T - 1, DA, Bc))], axis=1)
    acts = np.ascontiguousarray(acts.astype(bf))
    ones = np.ones((1, Bc), bf)
    return dict(tact=tact, acts=acts, ones=ones)


def build_nc(Bc, cf, T=20, gps_tig=False, gps_h_hi=False, loop_reps=None, skew=2, gates_bufs=3, tmps_bufs=5, g_exit_dve=False):
    """Build the Bass graph for one core (SPMD: all cores identical)."""
    nchunks = Bc // NC
    nsteps = T - 1
    nout = T - cf
    nc = bacc.Bacc(None)

    # --- DRAM params
    wshapes = dict(wz1k0=(49, 800), wz1k1=(72, 800), wz1k2=(128, 800),
                   wz2k0=(128, 800),
                   wz2k1=(85, 800), wz2k2=(72, 800), wz2k3=(128, 800),
                   wf1k0=(128, 200), wf1k1=(73, 200), wf1k2=(48, 200),
                   wf2k0=(128, 48), wf2k1=(72, 48), wf2k2=(1, 48))
    wext = {k: nc.declare_dram_parameter(k, list(s), BF, isOutput=False)
            for k, s in wshapes.items()}
    tact_e = nc.declare_dram_parameter("tact", [max(cf, 1), DT, Bc], BF, isOutput=False)
    ones_e = nc.declare_dram_parameter("ones", [1, Bc], BF, isOutput=False)
    acts_e = nc.declare_dram_parameter("acts", [nsteps, 2 * DA, Bc], BF, isOutput=False)
    out_e = nc.declare_dram_parameter("out", [nout, DT, Bc], BF, isOutput=True)

    with tile.TileContext(nc) as tc, contextlib.ExitStack() as ctx:
        wpool = ctx.enter_context(tc.tile_pool(name="w", bufs=1))
        state = ctx.enter_context(tc.tile_pool(name="state", bufs=1))
        gates = ctx.enter_context(tc.tile_pool(name="gates", bufs=gates_bufs))
        tmps = ctx.enter_context(tc.tile_pool(name="tmps", bufs=tmps_bufs))
        ps3 = ctx.enter_context(tc.tile_pool(name="ps3", bufs=2, space="PSUM"))
        ps1 = ctx.enter_context(tc.tile_pool(name="ps1", bufs=2, space="PSUM"))

        # weights -> SBUF
        wsb = {}
        for k, s in wshapes.items():
            wt = wpool.tile(list(s), BF, tag=k)
            nc.sync.dma_start(wt[:], wext[k][:])
            wsb[k] = wt

        # persistent state tiles
        xt1 = state.tile([49, Bc], BF, tag="xt1")    # [x1(48); ones(1)]
        h1hi = state.tile([HHI, Bc], BF, tag="h1hi")
        hs1 = state.tile([85, Bc], BF, tag="hs1")    # [h1_hi(72); act(6); state(6); ones]
        h1A = state.tile([HLO, Bc], BF, tag="h1A")
        h2A = state.tile([HLO, Bc], BF, tag="h2A")
        h2hi = state.tile([73, Bc], BF, tag="h2hi")  # [h2_hi(72); ones]
        c1lo = state.tile([HLO, Bc], F32, tag="c1lo")
        c1hi = state.tile([HHI, Bc], F32, tag="c1hi")
        c2lo = state.tile([HLO, Bc], F32, tag="c2lo")
        c2hi = state.tile([HHI, Bc], F32, tag="c2hi")

        # init
        for t_ in (c1lo, c1hi, c2lo, c2hi):
            nc.vector.memset(t_[:], 0.0)
        nc.vector.memset(h1A[:], 0.0)
        nc.vector.memset(h2A[:], 0.0)
        nc.vector.memset(h2hi[0:72, :], 0.0)
        nc.vector.memset(h1hi[:], 0.0)
        nc.sync.dma_start(xt1[48:49, :], ones_e[:])
        nc.sync.dma_start(hs1[84:85, :], ones_e[:])
        nc.sync.dma_start(h2hi[72:73, :], ones_e[:])
        ones_sb = state.tile([1, NC], BF, tag="ones_sb")
        nc.sync.dma_start(ones_sb[:], ones_e[:, 0:NC])
        if cf == 0:
            nc.vector.memset(xt1[0:48, :], 0.0)

        GOFF_LO = [0, 128, 256, 384]          # col offsets (x128) of i,f,o,g lo in w tiles
        GOFF_HI = [512, 584, 656, 728]        # i,f,o,g hi (x72)

        def z_mms(kpieces, ch, lstm):
            """Matmuls for one z of chunk ch -> (s3, s1) x (lo, hi) slabs."""
            cs = slice(ch * NC, (ch + 1) * NC)
            s3l = ps3.tile([HLO, 3 * NC], F32, tag="s3")
            s3h = ps3.tile([HHI, 3 * NC], F32, tag="s3")
            s1l = ps1.tile([HLO, NC], F32, tag="s1")
            s1h = ps1.tile([HHI, NC], F32, tag="s1")
            nk = len(kpieces)
            for gi in range(3):  # i, f, o -> s3
                for slab, offs, mw in ((s3l, GOFF_LO, HLO), (s3h, GOFF_HI, HHI)):
                    mo = offs[gi]
                    for ki, (wn, rhs) in enumerate(kpieces):
                        nc.tensor.matmul(
                            slab[:, gi * NC:(gi + 1) * NC],
                            wsb[wn][:, mo:mo + mw], rhs[:, cs],
                            start=(ki == 0), stop=(ki == nk - 1))
            for slab, offs, mw in ((s1l, GOFF_LO, HLO), (s1h, GOFF_HI, HHI)):
                mo = offs[3]
                for ki, (wn, rhs) in enumerate(kpieces):
                    nc.tensor.matmul(slab[:], wsb[wn][:, mo:mo + mw], rhs[:, cs],
                                     start=(ki == 0), stop=(ki == nk - 1))
            return s3l, s3h, s1l, s1h

        def lstm_elem(slabs, ch, clo, chi, hdst_lo, hdst_hi, extra_copy_dst):
            """Gate activations + cell update for one lstm, one chunk."""
            cs = slice(ch * NC, (ch + 1) * NC)
            s3l, s3h, s1l, s1h = slabs
            glo = gates.tile([HLO, 4 * NC], F32, tag="glo")
            ghi = gates.tile([HHI, 4 * NC], F32, tag="ghi")
            nc.scalar.activation(glo[:, 0:3 * NC], s3l[:], F.Sigmoid)
            nc.scalar.activation(ghi[:, 0:3 * NC], s3h[:], F.Sigmoid)
            if g_exit_dve:
                # DVE drains the g-slab; ACT tanh runs on the fast SBUF path
                nc.vector.tensor_copy(glo[:, 3 * NC:4 * NC], s1l[:])
                nc.vector.tensor_copy(ghi[:, 3 * NC:4 * NC], s1h[:])
            else:
                nc.scalar.activation(glo[:, 3 * NC:4 * NC], s1l[:], F.Tanh)
                nc.scalar.activation(ghi[:, 3 * NC:4 * NC], s1h[:], F.Tanh)
            for g, c_t, hdst in ((glo, clo, hdst_lo), (ghi, chi, hdst_hi)):
                p = g.shape[0]
                i_ = g[:, 0:NC]
                f_ = g[:, NC:2 * NC]
                o_ = g[:, 2 * NC:3 * NC]
                g_ = g[:, 3 * NC:4 * NC]
                c_sl = c_t[:, cs]
                t_ig = tmps.tile([p, NC], F32, tag=f"tmp{p}")
                t_fc = tmps.tile([p, NC], F32, tag=f"tmp{p}")
                if g_exit_dve:
                    t_g = tmps.tile([p, NC], F32, tag=f"tmp{p}")
                    nc.scalar.activation(t_g[:], g_, F.Tanh)
                    g_ = t_g[:]
                if gps_tig:
                    nc.gpsimd.tensor_tensor(t_ig[:], i_, g_, A.mult)
                else:
                    nc.vector.scalar_tensor_tensor(t_ig[:], i_, 1.0, g_, A.bypass, A.mult)
                nc.vector.scalar_tensor_tensor(t_fc[:], f_, 1.0, c_sl, A.bypass, A.mult)
                nc.vector.scalar_tensor_tensor(c_sl, t_ig[:], 1.0, t_fc[:], A.bypass, A.add)
                t_tc = tmps.tile([p, NC], F32, tag=f"tmp{p}")
                nc.scalar.activation(t_tc[:], c_sl, F.Tanh)
                is_hi = hdst is hdst_hi and extra_copy_dst is not None
                if g is ghi and gps_h_hi:
                    nc.gpsimd.tensor_tensor(hdst, o_, t_tc[:], A.mult)
                else:
                    nc.vector.scalar_tensor_tensor(hdst, o_, 1.0, t_tc[:], A.bypass, A.mult)
                if g is ghi and extra_copy_dst is not None:
                    nc.vector.tensor_copy(extra_copy_dst, hdst)

        def emit_lstm1(t, ch):
            cs = slice(ch * NC, (ch + 1) * NC)
            slabs = z_mms([("wz1k0", xt1), ("wz1k1", h1hi), ("wz1k2", h1A)], ch, 1)
            lstm_elem(slabs, ch, c1lo, c1hi,
                      h1A[:, cs], h1hi[:, cs], hs1[0:72, cs])

        def emit_lstm2(t, ch):
            cs = slice(ch * NC, (ch + 1) * NC)
            slabs = z_mms([("wz2k0", h1A), ("wz2k1", hs1),
                           ("wz2k2", h2hi[0:72, :]), ("wz2k3", h2A)], ch, 2)
            lstm_elem(slabs, ch, c2lo, c2hi, h2A[:, cs], h2hi[0:72, cs], None)

        def emit_fc(t, ch):
            cs = slice(ch * NC, (ch + 1) * NC)
            f1l = ps1.tile([HLO, NC], F32, tag="s1")
            f1h = ps1.tile([HHI, NC], F32, tag="s1")
            pieces = [("wf1k0", h2A), ("wf1k1", h2hi), ("wf1k2", xt1[0:48, :])]
            for ki, (wn, rhs) in enumerate(pieces):
                nc.tensor.matmul(f1l[:], wsb[wn][:, 0:HLO], rhs[:, cs],
                                 start=(ki == 0), stop=(ki == 2))
            for ki, (wn, rhs) in enumerate(pieces):
                nc.tensor.matmul(f1h[:], wsb[wn][:, HLO:H], rhs[:, cs],
                                 start=(ki == 0), stop=(ki == 2))
            o3A = gates.tile([HLO, NC], BF, tag="o3A")
            o3hi = gates.tile([HHI, NC], BF, tag="o3hi")
            nc.scalar.activation(o3A[:], f1l[:], F.Tanh)
            nc.scalar.activation(o3hi[:], f1h[:], F.Tanh)
            f2 = ps1.tile([DT, NC], F32, tag="s1")
            p2 = [("wf2k0", o3A[:]), ("wf2k1", o3hi[:]), ("wf2k2", ones_sb[:])]
            for ki, (wn, rhs) in enumerate(p2):
                nc.tensor.matmul(f2[:], wsb[wn][:, 0:DT], rhs,
                                 start=(ki == 0), stop=(ki == 2))
            nc.scalar.activation(xt1[0:48, cs], f2[:], F.Tanh)

        # --- main loop, software-pipelined: lstm2/fc trail lstm1 by one chunk
        loop_cm = tc.For_i(0, loop_reps, 1) if loop_reps else contextlib.nullcontext()
        with loop_cm:
            for t in range(nsteps):
                if t < cf:
                    nc.sync.dma_start(xt1[0:48, :], tact_e[t])
                nc.sync.dma_start(hs1[72:84, :], acts_e[t])
                fc_step = t >= cf - 1
                for ch in range(nchunks + skew):
                    if ch < nchunks:
                        emit_lstm1(t, ch)
                    if ch >= skew:
                        emit_lstm2(t, ch - skew)
                        if fc_step:
                            emit_fc(t, ch - skew)
                if fc_step:
                    nc.sync.dma_start(out_e[t - (cf - 1)], xt1[0:48, :])

    nc.finalize()
    return nc


def reorder_outputs(res_out, B, n_cores, nout):
    """[cores][nout, 48, Bc] bf16 -> [nout, B, 48] f32"""
    full = np.concatenate([np.transpose(np.float32(r), (0, 2, 1)) for r in res_out],
                          axis=1)
    return np.ascontiguousarray(full)


_BUILD_CACHE = {}


def kernel(tactiles, actions, Wih1, Whh1, bih1, bhh1, Wih2, Whh2, bih2, bhh2,
           W1, b1, W2, b2, context_frames):
    """Full-input entry point: shards across 8 NeuronCores, returns full output."""
    from concourse.bass_utils import run_bass_kernel_spmd

    tactiles = np.asarray(tactiles)
    actions = np.asarray(actions)
    cf = int(np.asarray(context_frames))
    T, B, _ = tactiles.shape
    n_cores = 8
    Bc = B // n_cores

    key = (Bc, cf, T)
    if key not in _BUILD_CACHE:
        _BUILD_CACHE[key] = build_nc(Bc=Bc, cf=cf, T=T)
    nc = _BUILD_CACHE[key]

    inp = dict(Wih1=Wih1, Whh1=Whh1, bih1=bih1, bhh1=bhh1, Wih2=Wih2, Whh2=Whh2,
               bih2=bih2, bhh2=bhh2, W1=W1, b1=b1, W2=W2, b2=b2)
    ws = prep_weights(inp)
    in_maps = []
    for core in range(n_cores):
        m = dict(ws)
        m.update(prep_core_inputs(tactiles, actions, cf, T, core, n_cores))
        in_maps.append(m)

    res = run_bass_kernel_spmd(nc, in_maps, core_ids=list(range(n_cores)))
    out = reorder_outputs([r["out"] for r in res.results], B, n_cores, T - cf)
    return out.astype(np.float32)



# revision 2
# speedup vs baseline: 1.0427x; 1.0427x over previous
"""Stage A rewrite: bf16 cell, K-packed z1/fc2, unified gate tiles, merged
elementwise ops, ping-pong step buffers.

Layouts (per core, Bc=4096, chunks of NC=512, feature-major [feat, batch]):
- xh1 (x2 ping-pong) [121, Bc] bf16: rows 0:48 x1, 48 ones, 49:121 h1_hi.
- h1A [128, Bc] bf16 (h1 rows 0:128).
- hs1 (x2 ping-pong) [85, Bc] bf16: rows 0:72 h1_hi copy, 72:84 acts, 84 ones.
- h2A [128, Bc], h2hi [73, Bc] (72 rows + ones) bf16.
- c1, c2 [128, 2*Bc] bf16: cols 0:Bc = H rows 0:128, Bc:2Bc = H rows 128:200
  (partitions 0:72).
- gates g1, g2 per chunk [128, 4096] bf16: per gate 1024 cols (lo 512 | hi 512),
  gate order i,f,o,g.
- PSUM slabs unchanged: s3l [128,1536], s3h [72,1536] (i,f,o), s1l/s1h (g).
"""
import sys as _sys
for _p in ("/opt/trn_rl_repo", "/root/.axon_site/_ro/trn_rl_repo"):
    if _p not in _sys.path:
        _sys.path.append(_p)

import numpy as np
import ml_dtypes
import contextlib

import concourse.bass as bass
import concourse.tile as tile
from concourse import bacc, mybir

F = mybir.ActivationFunctionType
A = mybir.AluOpType
BF = mybir.dt.bfloat16
F32 = mybir.dt.float32

H, DT, DA = 200, 48, 6
HLO, HHI = 128, 72
NC = 512


def perm_indices():
    # gate order i,f,o,g; each gate split lo(128)+hi(72)
    gr = {"i": 0, "f": 200, "o": 600, "g": 400}
    order = ["i", "f", "o", "g"]
    lo = np.concatenate([np.arange(gr[g], gr[g] + HLO) for g in order])
    hi = np.concatenate([np.arange(gr[g] + HLO, gr[g] + H) for g in order])
    return np.concatenate([lo, hi])


def prep_weights(inp):
    P = perm_indices()
    f32 = np.float32
    Wih1, Whh1 = f32(inp["Wih1"]), f32(inp["Whh1"])
    Wih2, Whh2 = f32(inp["Wih2"]), f32(inp["Whh2"])
    W1, W2 = f32(inp["W1"]), f32(inp["W2"])
    b1, b2 = f32(inp["b1"]), f32(inp["b2"])
    bb1 = f32(inp["bih1"]) + f32(inp["bhh1"])
    bb2 = f32(inp["bih2"]) + f32(inp["bhh2"])

    # z1 pieces: xh1=[h1hi(72); x1(48); ones] (121), h1A (128)
    wz1k0 = np.concatenate([Whh1[:, HLO:H], Wih1, bb1[:, None]], 1)[P].T  # [121,800]
    wz1k1 = Whh1[:, 0:HLO][P].T  # [128,800]

    # z2 pieces: h1A (128), hs1=[h1hi(72); act(6); state(6); ones] (85),
    # h2hi[0:72] (72), h2A (128)
    Wt = Wih2[:, H:H + DT]
    Wact = Wt[:, 0:6] + Wt[:, 12:18] + Wt[:, 24:30] + Wt[:, 36:42]
    Wsta = Wt[:, 6:12] + Wt[:, 18:24] + Wt[:, 30:36] + Wt[:, 42:48]
    wz2k0 = Wih2[:, 0:HLO][P].T
    wz2k1 = np.concatenate([Wih2[:, HLO:H], Wact, Wsta, bb2[:, None]], 1)[P].T  # [85,800]
    wz2k2 = Whh2[:, HLO:H][P].T  # [72,800]
    wz2k3 = Whh2[:, 0:HLO][P].T  # [128,800]

    # fc1: lp=[h2(200); x1(48)]: pieces h2A, h2hi73=[h2hi;ones], xh1[0:48]
    wf1k0 = W1[:, 0:HLO].T  # [128,200]
    wf1k1 = np.concatenate([W1[:, HLO:H], b1[:, None]], 1).T  # [73,200]
    wf1k2 = W1[:, H:H + DT].T  # [48,200]
    # fc2: pieces o3 (73 = o3hi(72)+ones), o3A (128)
    wf2k0 = W2[:, 0:HLO].T  # [128,48]
    wf2k1 = np.concatenate([W2[:, HLO:H], b2[:, None]], 1).T  # [73,48]

    ws = dict(wz1k0=wz1k0, wz1k1=wz1k1, wz2k0=wz2k0, wz2k1=wz2k1,
              wz2k2=wz2k2, wz2k3=wz2k3, wf1k0=wf1k0, wf1k1=wf1k1, wf1k2=wf1k2,
              wf2k0=wf2k0, wf2k1=wf2k1)
    return {k: np.ascontiguousarray(v.astype(ml_dtypes.bfloat16)) for k, v in ws.items()}


def prep_core_inputs(tactiles, actions, cf, T, core, n_cores):
    B = tactiles.shape[1]
    Bc = B // n_cores
    sl = slice(core * Bc, (core + 1) * Bc)
    bf = ml_dtypes.bfloat16
    tact = np.ascontiguousarray(
        np.transpose(tactiles[:cf, sl, :], (0, 2, 1)).astype(bf))
    at = np.transpose(actions[:, sl, :], (0, 2, 1))  # [T, 6, Bc]
    acts = np.concatenate(
        [at[1:T], np.broadcast_to(at[0:1], (T - 1, DA, Bc))], axis=1)
    acts = np.ascontiguousarray(acts.astype(bf))
    ones = np.ones((1, Bc), bf)
    return dict(tact=tact, acts=acts, ones=ones)


def build_nc(Bc, cf, T=20, skew=2):
    nchunks = Bc // NC
    nsteps = T - 1
    nout = T - cf
    nc = bacc.Bacc(None)

    wshapes = dict(wz1k0=(121, 800), wz1k1=(128, 800),
                   wz2k0=(128, 800), wz2k1=(85, 800), wz2k2=(72, 800),
                   wz2k3=(128, 800),
                   wf1k0=(128, 200), wf1k1=(73, 200), wf1k2=(48, 200),
                   wf2k0=(128, 48), wf2k1=(73, 48))
    wext = {k: nc.declare_dram_parameter(k, list(s), BF, isOutput=False)
            for k, s in wshapes.items()}
    tact_e = nc.declare_dram_parameter("tact", [max(cf, 1), DT, Bc], BF, isOutput=False)
    ones_e = nc.declare_dram_parameter("ones", [1, Bc], BF, isOutput=False)
    acts_e = nc.declare_dram_parameter("acts", [nsteps, 2 * DA, Bc], BF, isOutput=False)
    out_e = nc.declare_dram_parameter("out", [nout, DT, Bc], BF, isOutput=True)

    with tile.TileContext(nc) as tc, contextlib.ExitStack() as ctx:
        wpool = ctx.enter_context(tc.tile_pool(name="w", bufs=1))
        state = ctx.enter_context(tc.tile_pool(name="state", bufs=1))
        gates = ctx.enter_context(tc.tile_pool(name="gates", bufs=3))
        tmps = ctx.enter_context(tc.tile_pool(name="tmps", bufs=5))
        ps3 = ctx.enter_context(tc.tile_pool(name="ps3", bufs=2, space="PSUM"))
        ps1 = ctx.enter_context(tc.tile_pool(name="ps1", bufs=2, space="PSUM"))

        wsb = {}
        for k, s in wshapes.items():
            wt = wpool.tile(list(s), BF, tag=k)
            nc.sync.dma_start(wt[:], wext[k][:])
            wsb[k] = wt

        # persistent state
        xh1 = [state.tile([128, Bc], BF, tag=f"xh1_{p}") for p in range(2)]
        hs1 = [state.tile([85, Bc], BF, tag=f"hs1_{p}") for p in range(2)]
        h1A = state.tile([HLO, Bc], BF, tag="h1A")
        h2A = state.tile([HLO, Bc], BF, tag="h2A")
        h2hi = state.tile([73, Bc], BF, tag="h2hi")
        c1 = state.tile([HLO, 2 * Bc], BF, tag="c1")
        c2 = state.tile([HLO, 2 * Bc], BF, tag="c2")
        o3t = [state.tile([73, NC], BF, tag=f"o3_{p}") for p in range(2)]

        # init
        nc.vector.memset(c1[:], 0.0)
        nc.vector.memset(c2[:], 0.0)
        nc.vector.memset(h1A[:], 0.0)
        nc.vector.memset(h2A[:], 0.0)
        nc.vector.memset(h2hi[0:72, :], 0.0)
        for p in range(2):
            nc.vector.memset(xh1[p][0:72, :], 0.0)
            nc.vector.memset(hs1[p][0:72, :], 0.0)
            nc.sync.dma_start(xh1[p][120:121, :], ones_e[:])
            nc.sync.dma_start(hs1[p][84:85, :], ones_e[:])
            nc.sync.dma_start(o3t[p][72:73, :], ones_e[:, 0:NC])
        nc.sync.dma_start(h2hi[72:73, :], ones_e[:])
        if cf == 0:
            for p in range(2):
                nc.vector.memset(xh1[p][72:120, :], 0.0)

        GOFF_LO = [0, 128, 256, 384]   # i,f,o,g lo col offsets in w tiles
        GOFF_HI = [512, 584, 656, 728]

        def z_mms(kpieces, ch):
            """Gate matmuls for one z, chunk ch. Returns psum slabs."""
            cs = slice(ch * NC, (ch + 1) * NC)
            s3l = ps3.tile([HLO, 3 * NC], F32, tag="s3")
            s3h = ps3.tile([HHI, 3 * NC], F32, tag="s3")
            s1l = ps1.tile([HLO, NC], F32, tag="s1")
            s1h = ps1.tile([HHI, NC], F32, tag="s1")
            nk = len(kpieces)
            for gi in range(3):  # i, f, o
                for slab, offs, mw in ((s3l, GOFF_LO, HLO), (s3h, GOFF_HI, HHI)):
                    mo = offs[gi]
                    for ki, (wn, rhs) in enumerate(kpieces):
                        nc.tensor.matmul(
                            slab[:, gi * NC:(gi + 1) * NC],
                            wsb[wn][:, mo:mo + mw], rhs[:, cs],
                            start=(ki == 0), stop=(ki == nk - 1))
            for slab, offs, mw in ((s1l, GOFF_LO, HLO), (s1h, GOFF_HI, HHI)):
                mo = offs[3]
                for ki, (wn, rhs) in enumerate(kpieces):
                    nc.tensor.matmul(slab[:], wsb[wn][:, mo:mo + mw], rhs[:, cs],
                                     start=(ki == 0), stop=(ki == nk - 1))
            return s3l, s3h, s1l, s1h

        def cpair(c_t, ch):
            """3D AP covering (lo cols, hi cols) of chunk ch in a c tile."""
            r = c_t[:].rearrange("p (two b) -> p two b", two=2)
            return r[:, :, ch * NC:(ch + 1) * NC]

        def lstm_elem(slabs, ch, c_t, hdst_lo, hdst_hi, copy_dst):
            """Gate nonlinearities + cell update, unified gates tile.
            g layout: [i_lo f_lo o_lo | i_hi f_hi o_hi | g_lo g_hi] x NC."""
            s3l, s3h, s1l, s1h = slabs
            g = gates.tile([HLO, 8 * NC], BF, tag="g")
            gr = g[:].rearrange("p (blk b) -> p blk b", b=NC)
            nc.scalar.activation(g[:, 0:3 * NC], s3l[:], F.Sigmoid)
            nc.scalar.activation(g[0:HHI, 3 * NC:6 * NC], s3h[:], F.Sigmoid)
            nc.scalar.activation(g[:, 6 * NC:7 * NC], s1l[:], F.Tanh)
            nc.scalar.activation(g[0:HHI, 7 * NC:8 * NC], s1h[:], F.Tanh)
            t_ig = tmps.tile([HLO, 2 * NC], BF, tag="t_ig")
            t_fc = tmps.tile([HLO, 2 * NC], BF, tag="t_fc")
            t_tc = tmps.tile([HLO, 2 * NC], BF, tag="t_tc")
            cap = cpair(c_t, ch)
            c_lo = c_t[:, ch * NC:(ch + 1) * NC]
            c_hi = c_t[0:HHI, Bc + ch * NC:Bc + (ch + 1) * NC]
            # i*g
            nc.gpsimd.tensor_tensor(t_ig[:, 0:NC], g[:, 0:NC], g[:, 6 * NC:7 * NC], A.mult)
            nc.gpsimd.tensor_tensor(t_ig[0:HHI, NC:2 * NC], g[0:HHI, 3 * NC:4 * NC],
                                    g[0:HHI, 7 * NC:8 * NC], A.mult)
            # f*c
            nc.vector.tensor_tensor(t_fc[:, 0:NC], g[:, NC:2 * NC], c_lo, A.mult)
            nc.vector.tensor_tensor(t_fc[0:HHI, NC:2 * NC], g[0:HHI, 4 * NC:5 * NC], c_hi, A.mult)
            # c = ig + fc
            nc.vector.tensor_tensor(c_lo, t_ig[:, 0:NC], t_fc[:, 0:NC], A.add)
            nc.vector.tensor_tensor(c_hi, t_ig[0:HHI, NC:2 * NC], t_fc[0:HHI, NC:2 * NC], A.add)
            # tanh(c) merged (hi pad partitions stay zero from init)
            nc.scalar.activation(t_tc[:], cap, F.Tanh)
            nc.vector.tensor_tensor(hdst_lo, g[:, 2 * NC:3 * NC], t_tc[:, 0:NC], A.mult)
            nc.vector.tensor_tensor(hdst_hi, g[0:HHI, 5 * NC:6 * NC], t_tc[0:HHI, NC:2 * NC], A.mult)
            if copy_dst is not None:
                nc.vector.tensor_copy(copy_dst, hdst_hi)

        def emit_lstm1(t, ch):
            cs = slice(ch * NC, (ch + 1) * NC)
            xcur, xnxt = xh1[t % 2], xh1[(t + 1) % 2]
            slabs = z_mms([("wz1k0", xcur[0:121, :]), ("wz1k1", h1A)], ch)
            lstm_elem(slabs, ch, c1, h1A[:, cs], xnxt[0:72, cs],
                      hs1[t % 2][0:72, cs])

        def emit_lstm2(t, ch):
            cs = slice(ch * NC, (ch + 1) * NC)
            slabs = z_mms([("wz2k0", h1A), ("wz2k1", hs1[t % 2]),
                           ("wz2k2", h2hi[0:72, :]), ("wz2k3", h2A)], ch)
            lstm_elem(slabs, ch, c2, h2A[:, cs], h2hi[0:72, cs], None)

        def emit_fc(t, ch):
            cs = slice(ch * NC, (ch + 1) * NC)
            xcur, xnxt = x1t[t % 2], x1t[(t + 1) % 2]
            f1l = ps1.tile([HLO, NC], F32, tag="s1")
            f1h = ps1.tile([HHI, NC], F32, tag="s1")
            pieces = [("wf1k0", h2A), ("wf1k1", h2hi), ("wf1k2", xcur)]
            for ki, (wn, rhs) in enumerate(pieces):
                nc.tensor.matmul(f1l[:], wsb[wn][:, 0:HLO], rhs[:, cs],
                                 start=(ki == 0), stop=(ki == 2))
            for ki, (wn, rhs) in enumerate(pieces):
                nc.tensor.matmul(f1h[:], wsb[wn][:, HLO:H], rhs[:, cs],
                                 start=(ki == 0), stop=(ki == 2))
            o3 = o3t[ch % 2]
            o3A = gates.tile([HLO, NC], BF, tag="o3A")
            nc.scalar.activation(o3A[:], f1l[:], F.Tanh)
            nc.scalar.activation(o3[0:72, :], f1h[:], F.Tanh)
            f2 = ps1.tile([DT, NC], F32, tag="s1")
            p2 = [("wf2k0", o3A[:]), ("wf2k1", o3[:])]
            for ki, (wn, rhs) in enumerate(p2):
                nc.tensor.matmul(f2[:], wsb[wn][:, 0:DT], rhs,
                                 start=(ki == 0), stop=(ki == 1))
            nc.scalar.activation(xnxt[:, cs], f2[:], F.Tanh)
            if t + 1 < nsteps:
                nc.sync.dma_start(xh1[(t + 1) % 2][72:120, cs], xnxt[:, cs])

        for t in range(nsteps):
            if t < cf:
                nc.sync.dma_start(xh1[t % 2][72:120, :], tact_e[t])
                if t == cf - 1:
                    nc.sync.dma_start(x1t[t % 2][:], tact_e[t])
            nc.sync.dma_start(hs1[t % 2][72:84, :], acts_e[t])
            fc_step = t >= cf - 1
            for ch in range(nchunks + skew):
                if ch < nchunks:
                    emit_lstm1(t, ch)
                if ch >= skew:
                    emit_lstm2(t, ch - skew)
                    if fc_step:
                        emit_fc(t, ch - skew)
            if fc_step:
                nc.sync.dma_start(out_e[t - (cf - 1)], x1t[(t + 1) % 2][:])

    nc.finalize()
    return nc


def reorder_outputs(res_out, B, n_cores, nout):
    full = np.concatenate([np.transpose(np.float32(r), (0, 2, 1)) for r in res_out],
                          axis=1)
    return np.ascontiguousarray(full)


_BUILD_CACHE = {}


def kernel(tactiles, actions, Wih1, Whh1, bih1, bhh1, Wih2, Whh2, bih2, bhh2,
           W1, b1, W2, b2, context_frames):
    from concourse.bass_utils import run_bass_kernel_spmd

    tactiles = np.asarray(tactiles)
    actions = np.asarray(actions)
    cf = int(np.asarray(context_frames))
    T, B, _ = tactiles.shape
    n_cores = 8
    Bc = B // n_cores

    key = (Bc, cf, T)
    if key not in _BUILD_CACHE:
        _BUILD_CACHE[key] = build_nc(Bc=Bc, cf=cf, T=T)
    nc = _BUILD_CACHE[key]

    inp = dict(Wih1=Wih1, Whh1=Whh1, bih1=bih1, bhh1=bhh1, Wih2=Wih2, Whh2=Whh2,
               bih2=bih2, bhh2=bhh2, W1=W1, b1=b1, W2=W2, b2=b2)
    ws = prep_weights(inp)
    in_maps = []
    for core in range(n_cores):
        m = dict(ws)
        m.update(prep_core_inputs(tactiles, actions, cf, T, core, n_cores))
        in_maps.append(m)

    res = run_bass_kernel_spmd(nc, in_maps, core_ids=list(range(n_cores)))
    out = reorder_outputs([r["out"] for r in res.results], B, n_cores, T - cf)
    return out.astype(np.float32)
